# revision 3
# baseline (speedup 1.0000x reference)
"""Trainium2 Bass kernel for nn_CategoricalActivation (8-core data-parallel).

Reference semantics (per element x[s, b, h], column col=(b, h)):
    ss = x / (1 + |x|)                            # softsign
    boundaries b_c = x_raw[ind[c, col], col]      # 4 sampled rows per column
    counts = #{c : x > b_c} - 2.5
    cat  = cat_u[col] < 0.1
    ord  = (ord_u[col] < 0.7) & cat
    out  = ord ? 0.0 : (cat ? counts : ss)
(The "randomize_classes" remap is identically zero: counts values
{-2.5..1.5} never equal a class id 0..4, so remapped == 0 at ord cols.)

v4 design (per core; HBM-byte- and instruction-count-minimized):
  - ALL categorical columns (cat, ~10%) are compacted OUT of the bulk on
    the host: bulk is [S, KEEP=1872] bf16 (keep = non-cat columns only,
    padded).  Down from [S, 2048]: 8.6% fewer bulk bytes each way.  The
    softsign outputs for keep columns come back bf16; the host scatters
    them into the full output, writes 0.0 at ord columns and counts at
    catno (cat & ~ord) columns.
  - The bulk is staged in DEVICE TILE ORDER on the host: [4, 128, 7488]
    where granule g, partition p, (j*KEEP+w) maps to x row 512g+128j+p.
    Each in/out DMA is one fully contiguous 1.92 MB transfer (4 in + 4
    out DMAs total, vs 33 chunk DMAs in v3) - near line-rate DMA and ~3x
    fewer instructions/semaphores, which also shrinks the fixed
    preamble/teardown overhead (instruction-stream load + end-of-program
    semaphore drain chains measured ~12us in the v3 trace).
  - Compute granule is half an in-tile [128, 3744]: |x| on the DVE
    (bitwise_and 0x7FFF on an int16 view), r = 1/(1+|x|) on the Scalar
    engine (bias=1.0 folded into the activation), out = x*r in-place on
    the DVE.  Scalar-engine stream (~30us) stays off the critical path
    because loads arrive progressively (4 separate in DMAs).
  - counts compare RAW f32 values (order-equivalent to comparing
    softsign values; bf16 would create compare ties).  Host stages
    xce[KMAX=68, 4+S]: per catno column its 4 boundary values then the
    raw column.  4 fused compare passes on the DVE slack produce
    cnt = #{c: x > b_c} in {0..4}, returned bf16 (exact); host
    subtracts 2.5 while unsharding.
"""

import numpy as np

S = 2048
B = 16
H = 1024
NCORES = 8
BLOC = B // NCORES         # 2
C = BLOC * H               # 2048 columns per core
P = 128
KEEP = 1872                # padded non-cat (bulk) column slots per core
G = 4                      # in/out DMA granules (512 rows each)
JG = 4                     # row chunks per granule
W4 = JG * KEEP             # 7488 free elements per in-tile
W2 = W4 // 2               # 3744: compute granule (half tile)
KMAX = 68                  # padded catno (counts) column slots per core
NC5 = 5

_CACHE = {}


def _split_multi_waits(nc, scr_ap=None, max_waits=1):
    """This container's walrus rejects >1 sync-wait per instruction; hoist
    extra waits onto cheap same-engine carrier instructions inserted just
    before (tiny Memset on the pipelined engines - a Drain there would
    flush the pipe at ~0.4-2.4us - and Drain on the sequencer-only ones)."""
    import concourse.mybir as mybir

    memset_engines = {mybir.EngineType.DVE, mybir.EngineType.Pool}
    n_split = 0
    for f in nc.m.functions:
        for blk in f.blocks:
            insts = blk.instructions
            i = 0
            while i < len(insts):
                ins = insts[i]
                si = ins.sync_info
                if si is not None and len(si.on_wait) > max_waits:
                    waits = list(si.on_wait)
                    keep = waits[-max_waits:]
                    hoist = waits[:-max_waits]
                    for w in hoist:
                        if scr_ap is not None and ins.engine in memset_engines:
                            d = mybir.InstMemset(
                                name=f"I-{nc.next_id()}", mode="Const",
                                ins=[], outs=[scr_ap], constant=0)
                        else:
                            d = mybir.InstDrain(
                                name=f"I-{nc.next_id()}", ins=[], outs=[],
                                bass_is_fusable=False)
                        d.engine = ins.engine
                        d.sync_info = mybir.SyncInfo(on_wait=[w], on_update=[])
                        insts.insert(i, d)
                        i += 1
                        n_split += 1
                    si.on_wait = keep
                    ins.sync_info = si
                i += 1
    return n_split


def _act_unary(nc, out_ap, in_ap, func, bias=0.0):
    """One scalar-engine activation, float-immediate bias (bypasses the
    bass wrapper so Reciprocal is allowed; HW-measured ~1.2e-5 max err)."""
    import concourse.mybir as mybir

    eng = nc.scalar
    ins_ = [
        eng.lower_ap(in_ap),
        mybir.ImmediateValue(dtype=mybir.dt.float32, value=float(bias)),
        mybir.ImmediateValue(dtype=mybir.dt.float32, value=1.0),
        mybir.ImmediateValue(dtype=mybir.dt.float32, value=0.0),
    ]
    return eng.add_instruction(
        mybir.InstActivation(
            name=nc.get_next_instruction_name(),
            func=func,
            ins=ins_,
            outs=[eng.lower_ap(out_ap)],
        )
    )


def _build_program():
    import contextlib

    import concourse.bass as bass
    import concourse.tile as tile
    from concourse import mybir

    A = mybir.AluOpType
    F = mybir.ActivationFunctionType
    f32 = mybir.dt.float32
    bf16 = mybir.dt.bfloat16
    i16 = mybir.dt.int16
    i32 = mybir.dt.int32

    nc = bass.Bass()
    x_in = nc.dram_tensor("x", [G, P, W4], bf16, kind="ExternalInput")
    xce_in = nc.dram_tensor("xce", [KMAX, 4 + S], f32, kind="ExternalInput")
    out_d = nc.dram_tensor("out", [G, P, W4], bf16, kind="ExternalOutput")
    cnt_d = nc.dram_tensor("cnt", [KMAX, S], bf16, kind="ExternalOutput")

    with tile.TileContext(nc) as tc:
        with contextlib.ExitStack() as ctx:
            singles = ctx.enter_context(tc.tile_pool(name="singles", bufs=1))
            xp = ctx.enter_context(tc.tile_pool(name="xp", bufs=G))
            ap_ = ctx.enter_context(tc.tile_pool(name="ap", bufs=2))
            rp = ctx.enter_context(tc.tile_pool(name="rp", bufs=2))

            scr = singles.tile([1, 8], i32, name="scr")
            nc.vector.memset(scr, 0)

            # issue every in-DMA upfront on SP (single HWDGE FIFO): the
            # input streams at line rate, compute never gates a load
            xts = []
            for g in range(G):
                xt = xp.tile([P, W4], bf16, tag="xt", name=f"xt{g}")
                nc.sync.dma_start(out=xt, in_=x_in[g, :, :])
                xts.append(xt)
                if g == 0:
                    xce = singles.tile([KMAX, 4 + S], f32)
                    nc.sync.dma_start(out=xce, in_=xce_in[:, :])

            cnt = singles.tile([KMAX, S], f32)
            cntb = singles.tile([KMAX, S], bf16)

            def count_pass(c):
                # one boundary compare over the catno columns (DVE slack)
                if c == 0:
                    nc.vector.tensor_scalar(
                        out=cnt, in0=xce[:, 4:], scalar1=xce[:, 0:1],
                        scalar2=None, op0=A.is_gt)
                else:
                    nc.vector.scalar_tensor_tensor(
                        out=(cntb if c == 3 else cnt), in0=xce[:, 4:],
                        scalar=xce[:, c:c + 1], in1=cnt,
                        op0=A.is_gt, op1=A.add)

            for cg in range(2 * G):
                g, h = divmod(cg, 2)
                xv = xts[g][:, h * W2:(h + 1) * W2]
                xvi = xts[g].bitcast(i16)[:, h * W2:(h + 1) * W2]
                absx = ap_.tile([P, W2], bf16, tag="absx", name="absx")
                nc.vector.tensor_scalar(out=absx.bitcast(i16),
                                        in0=xvi,
                                        scalar1=0x7FFF, scalar2=None,
                                        op0=A.bitwise_and)
                ract = rp.tile([P, W2], bf16, tag="ract", name="ract")
                _act_unary(nc, ract[:, :], absx[:, :], F.Reciprocal, bias=1.0)
                nc.vector.tensor_tensor(out=xv, in0=xv, in1=ract, op=A.mult)
                if h == 1:
                    # whole granule computed -> one contiguous 1.9MB store
                    nc.sync.dma_start(out=out_d[g, :, :], in_=xts[g])
                if 1 <= cg <= 4:
                    count_pass(cg - 1)
                    if cg == 4:
                        nc.sync.dma_start(out=cnt_d[:, :], in_=cntb)

    _split_multi_waits(nc, scr_ap=nc.vector.lower_ap(scr[0:1, 0:1]))
    return nc


def _stage_bulk(xk):
    """[S, KEEP] f32 -> device tile order [G, P, JG*KEEP] bf16
    (granule g, partition p, segment j: row 512g + 128j + p)."""
    import ml_dtypes
    v = xk.reshape(G, JG, P, KEEP).transpose(0, 2, 1, 3).reshape(G, P, W4)
    return np.ascontiguousarray(v).astype(ml_dtypes.bfloat16)


def _unstage_bulk(ob):
    """[G, P, JG*KEEP] bf16 -> [S, KEEP] f32."""
    v = np.asarray(ob).astype(np.float32)
    return v.reshape(G, P, JG, KEEP).transpose(0, 2, 1, 3).reshape(S, KEEP)


def kernel(x, ind, cat_u, ord_u, perm, num_classes):
    from concourse.bass_utils import run_bass_kernel_spmd

    assert int(num_classes) == NC5
    x = np.ascontiguousarray(x, dtype=np.float32)
    ind = np.ascontiguousarray(ind, dtype=np.int32)
    cat_u = np.asarray(cat_u, dtype=np.float32)
    ord_u = np.asarray(ord_u, dtype=np.float32)
    assert x.shape == (S, B, H) and ind.shape == (4, B, H)

    cat = cat_u < np.float32(0.1)
    ordm = (ord_u < np.float32(0.7)) & cat
    catno = cat & ~ordm
    in_maps = []
    keep_lists = []
    cat_lists = []
    for m in range(NCORES):
        bs = slice(BLOC * m, BLOC * (m + 1))
        xm = x[:, bs, :].reshape(S, C)
        indm = ind[:, bs, :].reshape(4, C)
        kcols = np.nonzero(~cat[bs].reshape(C))[0].astype(np.int32)
        ccols = np.nonzero(catno[bs].reshape(C))[0].astype(np.int32)
        nk, kc = len(kcols), len(ccols)
        assert nk <= KEEP, f"core {m}: {nk} keep columns exceed KEEP"
        assert kc <= KMAX, f"core {m}: {kc} catno columns exceed KMAX"
        keep_lists.append(kcols)
        cat_lists.append(ccols)
        xk = np.zeros((S, KEEP), np.float32)
        xk[:, :nk] = xm[:, kcols]
        xce = np.zeros((KMAX, 4 + S), np.float32)
        xce[:kc, 4:] = xm[:, ccols].T
        xce[:kc, 0:4] = xm[indm[:, ccols], ccols].T
        in_maps.append({"x": _stage_bulk(xk), "xce": xce})

    if "nc" not in _CACHE:
        _CACHE["nc"] = _build_program()
    res = run_bass_kernel_spmd(_CACHE["nc"], in_maps,
                               core_ids=list(range(NCORES)))
    out = np.empty((S, B, H), np.float32)
    for m in range(NCORES):
        bs = slice(BLOC * m, BLOC * (m + 1))
        om = np.zeros((S, C), np.float32)
        kcols, ccols = keep_lists[m], cat_lists[m]
        ok = _unstage_bulk(res.results[m]["out"])
        om[:, kcols] = ok[:, :len(kcols)]
        if len(ccols):
            cm = np.asarray(res.results[m]["cnt"][:len(ccols)])
            om[:, ccols] = cm.astype(np.float32).T - np.float32(2.5)
        out[:, bs, :] = om.reshape(S, BLOC, H)
    return out
